# revision 23
# baseline (speedup 1.0000x reference)
"""Multi-head causal attention (B=2, S=2048, D=1024, H=16) on 8 TRN2 cores.

Sharding: core = (batch b = core//4, head-group g = core%4). Each core
computes 4 heads of one batch end-to-end (QKV projections for its head
slice, causal attention, its partial contribution to the output
projection). Host sums the 4 partial outputs per batch and adds the bias.

Device algorithm (per core), all matmuls in bf16 with f32 PSUM accum:
  qT/kT [dloc=256, S] = Wslice @ x.T   (x.T host-tiled to [IT,NCH,128,512]
                                        so every DMA tile is contiguous)
  V     [S, dloc]     (+ ones column per head for the softmax denominator)
  attention runs per (q-chunk of 512, head-PAIR):
    sT[k,q] for both heads of the pair -> one 2-bank PSUM tile
    attnT = exp(sT * 1/8) in ONE strided ScalarE op per k-tile pair
            (causal: k-tiles above the diagonal skipped, diagonal tiles
             use a q-subrange plus a 0/1 mask multiply)
    per head: AT_aug [65, q] = sum_k V_aug.T @ attnT          -> PSUM
              row 64 = softmax denominator l; AT = AT * bcast(1/l)
              (l broadcast over partitions via GpSimd, 1/l via fast
               approx reciprocal staged to partition 0)
  out_partial [S, 1024] = AT.T-free matmul with the Wo slice, f32 out.
Q/K/V projections for upcoming chunks and the Wo projection for finished
chunks are emitted interleaved with attention so the PE always has dense
independent work while ScalarE exp catches up (keeps the HAM clock-gate
warm). Input loads ride the sync HWDGE ring; V loads and output stores
ride the GpSimd SWDGE ring; weights ride the scalar HWDGE ring.
"""

import numpy as np
import ml_dtypes

D_MODEL = 1024
NUM_HEADS = 16
HEAD_DIM = 64
B = 2
S = 2048
N_CORES = 8
GROUPS = 4                 # head-groups (cores per batch)
HPC = NUM_HEADS // GROUPS  # 4 heads per core
DLOC = HPC * HEAD_DIM      # 256 local projection dims
P = 128
SCH = 512                  # q/s chunk
NCH = S // SCH             # 4
KT = S // P                # 16 k-tiles
IT = D_MODEL // P          # 8 contraction tiles
MB = DLOC // P             # 2 m-blocks

_CACHE = {}


def _build():
    import concourse.bass as bass
    import concourse.tile as tile
    from concourse import bacc, mybir

    F32 = mybir.dt.float32
    BF16 = mybir.dt.bfloat16

    nc = bacc.Bacc("TRN2", target_bir_lowering=False, debug=False,
                   num_devices=N_CORES)

    # inputs host-tiled: [IT, NCH, 128, 512] so each (r, c) tile is one
    # contiguous 128KB DMA
    xq = nc.dram_tensor("xq_t", [IT, NCH, P, SCH], BF16, kind="ExternalInput")
    xk = nc.dram_tensor("xk_t", [IT, NCH, P, SCH], BF16, kind="ExternalInput")
    xv = nc.dram_tensor("xv_t", [IT, NCH, P, SCH], BF16, kind="ExternalInput")
    wq = nc.dram_tensor("wq_t", [D_MODEL, DLOC], BF16, kind="ExternalInput")
    wk = nc.dram_tensor("wk_t", [D_MODEL, DLOC], BF16, kind="ExternalInput")
    wv = nc.dram_tensor("wv_t", [D_MODEL, DLOC], BF16, kind="ExternalInput")
    wo = nc.dram_tensor("wo_t", [DLOC, D_MODEL], BF16, kind="ExternalInput")
    mk = nc.dram_tensor("mask", [P, 4 * SCH], BF16, kind="ExternalInput")
    outp = nc.dram_tensor("outp", [S, D_MODEL], F32, kind="ExternalOutput")

    Exp = mybir.ActivationFunctionType.Exp

    with tile.TileContext(nc) as tc:
        with (
            tc.tile_pool(name="const", bufs=1) as constp,
            tc.tile_pool(name="persist", bufs=1) as pers,
            tc.tile_pool(name="inp", bufs=24) as inp,
            tc.tile_pool(name="attn", bufs=18) as attnp,
            tc.tile_pool(name="small", bufs=4) as small,
            tc.tile_pool(name="ostage", bufs=6) as ostage,
            tc.tile_pool(name="psA", bufs=2, space="PSUM") as psA,
            tc.tile_pool(name="psS", bufs=2, space="PSUM") as psS,
            tc.tile_pool(name="psO", bufs=2, space="PSUM") as psO,
        ):
            # ---- constants / persistent tensors ----
            wq_sb = constp.tile([P, IT, DLOC], BF16)
            wk_sb = constp.tile([P, IT, DLOC], BF16)
            wv_sb = constp.tile([P, IT, DLOC], BF16)
            wo_sb = constp.tile([P, MB, D_MODEL], BF16)
            mk_sb = constp.tile([P, 4 * SCH], BF16)

            qT_sb = pers.tile([P, MB, S], BF16)
            kT_sb = pers.tile([P, MB, S], BF16)
            v_sb = pers.tile([P, KT, HPC, HEAD_DIM + 1], BF16)
            atn_sb = pers.tile([P, MB, S], BF16)

            # DMA priority: wq/wk feed the very first matmuls.
            for r in range(IT):
                nc.scalar.dma_start(wq_sb[:, r, :], wq[r * P:(r + 1) * P, :])
            for r in range(IT):
                nc.scalar.dma_start(wk_sb[:, r, :], wk[r * P:(r + 1) * P, :])
            nc.scalar.dma_start(mk_sb[:], mk[:])
            nc.scalar.dma_start(wv_sb[:], wv[:].rearrange("(r p) d -> p r d", p=P))
            nc.scalar.dma_start(wo_sb[:], wo[:].rearrange("(m p) o -> p m o", p=P))
            nc.vector.memset(v_sb[:, :, :, HEAD_DIM:HEAD_DIM + 1], 1.0)

            def qk_proj(c, parts=(0, 1)):
                # Q and K projections for s-chunk c (part 0 = Q, 1 = K)
                srcs = ((xq, wq_sb, qT_sb), (xk, wk_sb, kT_sb))
                for part in parts:
                    x_dram, w_sb, dst = srcs[part]
                    ps = [psA.tile([P, SCH], F32, tag="psA", name=f"psqk{m}")
                          for m in range(MB)]
                    for r in range(IT):
                        xt = inp.tile([P, SCH], BF16, tag="inp")
                        nc.sync.dma_start(xt[:], x_dram[r, c])
                        for m in range(MB):
                            nc.tensor.matmul(
                                ps[m][:], w_sb[:, r, m * P:(m + 1) * P], xt[:],
                                start=(r == 0), stop=(r == IT - 1))
                    for m in range(MB):
                        nc.vector.tensor_copy(dst[:, m, c * SCH:(c + 1) * SCH],
                                              ps[m][:])

            _vx = {}

            def v_proj(c, js=(0, 1, 2, 3)):
                # V projection for s-tiles 4c+js; one PSUM slot per pass,
                # the 8 input tiles stay live across the passes.
                if c not in _vx:
                    xts = []
                    for r in range(IT):
                        xt = inp.tile([P, SCH], BF16, tag="inp",
                                      name=f"xv{c}_{r}")
                        nc.sync.dma_start(xt[:], xv[r, c])
                        xts.append(xt)
                    _vx[c] = xts
                xts = _vx[c]
                for j in js:
                    ps = psA.tile([P, DLOC], F32, tag="psA", name="psv")
                    for r in range(IT):
                        nc.tensor.matmul(
                            ps[:], xts[r][:, j * P:(j + 1) * P], wv_sb[:, r, :],
                            start=(r == 0), stop=(r == IT - 1))
                    nc.vector.tensor_copy(
                        v_sb[:, 4 * c + j, :, 0:HEAD_DIM],
                        ps[:].rearrange("p (h d) -> p h d", h=HPC))

            def wo_proj(c, ts=(0, 1, 2, 3)):
                # output projection for s-tiles 4c+ts
                for t in [4 * c + i for i in ts]:
                    for oc in range(D_MODEL // SCH):
                        ps_o = psA.tile([P, SCH], F32, tag="psA", name="pso")
                        for m in range(MB):
                            nc.tensor.matmul(
                                ps_o[:], atn_sb[:, m, t * P:(t + 1) * P],
                                wo_sb[:, m, oc * SCH:(oc + 1) * SCH],
                                start=(m == 0), stop=(m == MB - 1))
                        ot = ostage.tile([P, SCH], F32, tag="ot")
                        nc.vector.tensor_copy(ot[:], ps_o[:])
                        nc.gpsimd.dma_start(
                            outp[t * P:(t + 1) * P, oc * SCH:(oc + 1) * SCH], ot[:])

            def norm_head(h, c, ps_at):
                # AT[0:64] *= broadcast(1/l);  l = ps_at row 64.
                # approx_fast mishandles partition-offset inputs: stage the
                # l row to partition 0 first.
                m, po = h // 2, (h % 2) * HEAD_DIM
                lrow = small.tile([1, SCH], F32, tag="lrow")
                nc.vector.tensor_copy(lrow[:], ps_at[HEAD_DIM:HEAD_DIM + 1, :])
                linv = small.tile([1, SCH], F32, tag="linv")
                nc.vector.reciprocal_approx_fast(out=linv[:], in_=lrow[:])
                lbc = small.tile([HEAD_DIM, SCH], F32, tag="lbc")
                nc.gpsimd.partition_broadcast(lbc[:], linv[:])
                nc.vector.tensor_mul(
                    atn_sb[po:po + HEAD_DIM, m, c * SCH:(c + 1) * SCH],
                    ps_at[0:HEAD_DIM, :], lbc[:])

            qk_proj(0)
            v_proj(0)

            # ---- attention: chunk-major, head pairs, PE filler interleaved ----
            for c in range(NCH):
                nkt = 4 * (c + 1)  # causal: k-tiles 0..nkt-1

                def qoff(kt):
                    # diagonal k-tile j only needs q in [128j, 512)
                    return max(kt - 4 * c, 0) * P

                for hp in range(HPC // 2):
                    atts = []
                    for kt in range(nkt):
                        qo = qoff(kt)
                        ps_s = psS.tile([P, 2, SCH], F32, tag="psS")
                        for hh in range(2):
                            h = 2 * hp + hh
                            m, po = h // 2, (h % 2) * HEAD_DIM
                            nc.tensor.matmul(
                                ps_s[:, hh, qo:],
                                kT_sb[po:po + HEAD_DIM, m, kt * P:(kt + 1) * P],
                                qT_sb[po:po + HEAD_DIM, m,
                                      c * SCH + qo:(c + 1) * SCH],
                                start=True, stop=True)
                        att = attnp.tile([P, 2, SCH], BF16, tag="attn")
                        nc.scalar.activation(att[:, :, qo:],
                                             ps_s[:, :, qo:], Exp, scale=0.125)
                        j = kt - 4 * c
                        if j >= 0:  # diagonal tiles: apply causal mask
                            for hh in range(2):
                                nc.vector.tensor_mul(
                                    att[:, hh, qo:], att[:, hh, qo:],
                                    mk_sb[:, j * SCH + qo:(j + 1) * SCH])
                        atts.append(att)
                    for hh in range(2):
                        h = 2 * hp + hh
                        ps_at = psO.tile([HEAD_DIM + 1, SCH], F32, tag="psO",
                                         name="ps_at")
                        for kt in range(nkt):
                            qo = qoff(kt)
                            nc.tensor.matmul(
                                ps_at[:, qo:], v_sb[:, kt, h, :],
                                atts[kt][:, hh, qo:],
                                start=(kt == 0), stop=(kt == nkt - 1))
                        norm_head(h, c, ps_at)

                    # PE filler between head pairs / chunks: projections for
                    # upcoming chunks + output projection for finished ones.
                    if hp == 0:
                        if c + 1 < NCH:
                            qk_proj(c + 1)
                    else:
                        if c + 1 < NCH:
                            v_proj(c + 1)
                        if c >= 1:
                            wo_proj(c - 1)
            wo_proj(NCH - 1)

    nc.compile()
    return nc


def _get_nc():
    if "nc" not in _CACHE:
        _CACHE["nc"] = _build()
    return _CACHE["nc"]


def _mask_const():
    # mask[j][k, q] = 1.0 iff q >= j*128 + k, for diagonal k-tile offset j
    q = np.arange(SCH)[None, :]
    k = np.arange(P)[:, None]
    blocks = [(q >= j * P + k).astype(ml_dtypes.bfloat16) for j in range(4)]
    return np.concatenate(blocks, axis=1)  # [128, 2048]


def _tile_xt(x_t):
    # [D_MODEL, S] -> [IT, NCH, 128, 512] contiguous tiles
    return np.ascontiguousarray(
        x_t.reshape(IT, P, NCH, SCH).transpose(0, 2, 1, 3))


def kernel(query, key, value, mask, Wq, Wk, Wv, Wo, bo):
    from concourse.bass_utils import run_bass_kernel_spmd

    nc = _get_nc()
    bf = ml_dtypes.bfloat16

    xq_t = [_tile_xt(np.asarray(query)[b].T.astype(bf)) for b in range(B)]
    xk_t = [_tile_xt(np.asarray(key)[b].T.astype(bf)) for b in range(B)]
    xv_t = [_tile_xt(np.asarray(value)[b].T.astype(bf)) for b in range(B)]
    WqT = np.ascontiguousarray(np.asarray(Wq).T).astype(bf)  # [D, D] cols = out dim
    WkT = np.ascontiguousarray(np.asarray(Wk).T).astype(bf)
    WvT = np.ascontiguousarray(np.asarray(Wv).T).astype(bf)
    WoT = np.ascontiguousarray(np.asarray(Wo).T).astype(bf)
    mk = _mask_const()

    in_maps = []
    for core in range(N_CORES):
        b, g = core // GROUPS, core % GROUPS
        hsl = slice(g * DLOC, (g + 1) * DLOC)
        in_maps.append({
            "xq_t": xq_t[b], "xk_t": xk_t[b], "xv_t": xv_t[b],
            "wq_t": np.ascontiguousarray(WqT[:, hsl]),
            "wk_t": np.ascontiguousarray(WkT[:, hsl]),
            "wv_t": np.ascontiguousarray(WvT[:, hsl]),
            "wo_t": np.ascontiguousarray(WoT[hsl, :]),
            "mask": mk,
        })

    res = run_bass_kernel_spmd(nc, in_maps, core_ids=list(range(N_CORES)))
    _CACHE["last_result"] = res

    out = np.zeros((B, S, D_MODEL), np.float32)
    for core in range(N_CORES):
        out[core // GROUPS] += res.results[core]["outp"]
    out += np.asarray(bo, np.float32)[None, None, :]
    return out


# revision 25
# speedup vs baseline: 1.1755x; 1.1755x over previous
"""Multi-head causal attention (B=2, S=2048, D=1024, H=16) on 8 TRN2 cores.

Sharding: core = (batch b = core//4, head-group g = core%4). Each core
computes 4 heads of one batch end-to-end (QKV projections for its head
slice, causal attention, its partial contribution to the output
projection). Host sums the 4 partial outputs per batch and adds the bias.

Device algorithm (per core), all matmuls in bf16 with f32 PSUM accum:
  qT/kT [dloc=256, S] = Wslice @ x.T   (x.T host-tiled to [IT,NCH,128,512]
                                        so every DMA tile is contiguous)
  V     [S, dloc]     (+ ones column per head for the softmax denominator)
  attention runs per (q-chunk of 512, head-PAIR):
    sT[k,q] for both heads of the pair -> one 2-bank PSUM tile
    attnT = exp(sT * 1/8) in ONE strided ScalarE op per k-tile pair
            (causal: k-tiles above the diagonal skipped, diagonal tiles
             use a q-subrange plus a 0/1 mask multiply)
    per head: AT_aug [65, q] = sum_k V_aug.T @ attnT          -> PSUM
              row 64 = softmax denominator l; AT = AT * bcast(1/l)
              (l broadcast over partitions via GpSimd, 1/l via fast
               approx reciprocal staged to partition 0)
  out_partial [S, 1024] = AT.T-free matmul with the Wo slice, f32 out.
Q/K/V projections for upcoming chunks and the Wo projection for finished
chunks are emitted interleaved with attention so the PE always has dense
independent work while ScalarE exp catches up (keeps the HAM clock-gate
warm). Input loads ride the sync HWDGE ring; output stores ride the
GpSimd SWDGE ring; weights ride the scalar HWDGE ring.

The device kernel assumes the causal (lower-triangular) mask the
reference constructs; kernel() verifies that and falls back to an exact
numpy implementation for any other mask.
"""

import numpy as np
import ml_dtypes

D_MODEL = 1024
NUM_HEADS = 16
HEAD_DIM = 64
B = 2
S = 2048
N_CORES = 8
GROUPS = 4                 # head-groups (cores per batch)
HPC = NUM_HEADS // GROUPS  # 4 heads per core
DLOC = HPC * HEAD_DIM      # 256 local projection dims
P = 128
SCH = 512                  # q/s chunk
NCH = S // SCH             # 4
KT = S // P                # 16 k-tiles
IT = D_MODEL // P          # 8 contraction tiles
MB = DLOC // P             # 2 m-blocks

_CACHE = {}


def _build():
    import concourse.bass as bass
    import concourse.tile as tile
    from concourse import bacc, mybir

    F32 = mybir.dt.float32
    BF16 = mybir.dt.bfloat16

    nc = bacc.Bacc("TRN2", target_bir_lowering=False, debug=False,
                   num_devices=N_CORES)

    # inputs host-tiled: [IT, NCH, 128, 512] so each (r, c) tile is one
    # contiguous 128KB DMA
    xq = nc.dram_tensor("xq_t", [IT, NCH, P, SCH], BF16, kind="ExternalInput")
    xk = nc.dram_tensor("xk_t", [IT, NCH, P, SCH], BF16, kind="ExternalInput")
    xv = nc.dram_tensor("xv_t", [IT, NCH, P, SCH], BF16, kind="ExternalInput")
    wq = nc.dram_tensor("wq_t", [D_MODEL, DLOC], BF16, kind="ExternalInput")
    wk = nc.dram_tensor("wk_t", [D_MODEL, DLOC], BF16, kind="ExternalInput")
    wv = nc.dram_tensor("wv_t", [D_MODEL, DLOC], BF16, kind="ExternalInput")
    wo = nc.dram_tensor("wo_t", [DLOC, D_MODEL], BF16, kind="ExternalInput")
    mk = nc.dram_tensor("mask", [P, 4 * SCH], BF16, kind="ExternalInput")
    outp = nc.dram_tensor("outp", [S, D_MODEL], F32, kind="ExternalOutput")

    Exp = mybir.ActivationFunctionType.Exp

    with tile.TileContext(nc) as tc:
        with (
            tc.tile_pool(name="const", bufs=1) as constp,
            tc.tile_pool(name="persist", bufs=1) as pers,
            tc.tile_pool(name="inp", bufs=24) as inp,
            tc.tile_pool(name="attn", bufs=18) as attnp,
            tc.tile_pool(name="small", bufs=4) as small,
            tc.tile_pool(name="ostage", bufs=6) as ostage,
            tc.tile_pool(name="psA", bufs=2, space="PSUM") as psA,
            tc.tile_pool(name="psS", bufs=2, space="PSUM") as psS,
            tc.tile_pool(name="psO", bufs=2, space="PSUM") as psO,
        ):
            # ---- constants / persistent tensors ----
            wq_sb = constp.tile([P, IT, DLOC], BF16)
            wk_sb = constp.tile([P, IT, DLOC], BF16)
            wv_sb = constp.tile([P, IT, DLOC], BF16)
            wo_sb = constp.tile([P, MB, D_MODEL], BF16)
            mk_sb = constp.tile([P, 4 * SCH], BF16)

            qT_sb = pers.tile([P, MB, S], BF16)
            kT_sb = pers.tile([P, MB, S], BF16)
            v_sb = pers.tile([P, KT, HPC, HEAD_DIM + 1], BF16)
            atn_sb = pers.tile([P, MB, S], BF16)

            # DMA priority: wq/wk feed the very first matmuls.
            for r in range(IT):
                nc.scalar.dma_start(wq_sb[:, r, :], wq[r * P:(r + 1) * P, :])
            for r in range(IT):
                nc.scalar.dma_start(wk_sb[:, r, :], wk[r * P:(r + 1) * P, :])
            nc.scalar.dma_start(mk_sb[:], mk[:])
            nc.scalar.dma_start(wv_sb[:], wv[:].rearrange("(r p) d -> p r d", p=P))
            nc.scalar.dma_start(wo_sb[:], wo[:].rearrange("(m p) o -> p m o", p=P))
            nc.vector.memset(v_sb[:, :, :, HEAD_DIM:HEAD_DIM + 1], 1.0)

            def qk_proj(c, parts=(0, 1)):
                # Q and K projections for s-chunk c (part 0 = Q, 1 = K)
                srcs = ((xq, wq_sb, qT_sb), (xk, wk_sb, kT_sb))
                for part in parts:
                    x_dram, w_sb, dst = srcs[part]
                    ps = [psA.tile([P, SCH], F32, tag="psA", name=f"psqk{m}")
                          for m in range(MB)]
                    for r in range(IT):
                        xt = inp.tile([P, SCH], BF16, tag="inp")
                        nc.sync.dma_start(xt[:], x_dram[r, c])
                        for m in range(MB):
                            nc.tensor.matmul(
                                ps[m][:], w_sb[:, r, m * P:(m + 1) * P], xt[:],
                                start=(r == 0), stop=(r == IT - 1))
                    for m in range(MB):
                        nc.vector.tensor_copy(dst[:, m, c * SCH:(c + 1) * SCH],
                                              ps[m][:])

            _vx = {}

            def v_proj(c, js=(0, 1, 2, 3)):
                # V projection for s-tiles 4c+js; one PSUM slot per pass,
                # the 8 input tiles stay live across the passes.
                if c not in _vx:
                    xts = []
                    for r in range(IT):
                        xt = inp.tile([P, SCH], BF16, tag="inp",
                                      name=f"xv{c}_{r}")
                        nc.sync.dma_start(xt[:], xv[r, c])
                        xts.append(xt)
                    _vx[c] = xts
                xts = _vx[c]
                for j in js:
                    ps = psA.tile([P, DLOC], F32, tag="psA", name="psv")
                    for r in range(IT):
                        nc.tensor.matmul(
                            ps[:], xts[r][:, j * P:(j + 1) * P], wv_sb[:, r, :],
                            start=(r == 0), stop=(r == IT - 1))
                    nc.vector.tensor_copy(
                        v_sb[:, 4 * c + j, :, 0:HEAD_DIM],
                        ps[:].rearrange("p (h d) -> p h d", h=HPC))

            def wo_proj(c, ts=(0, 1, 2, 3)):
                # output projection for s-tiles 4c+ts
                for t in [4 * c + i for i in ts]:
                    for oc in range(D_MODEL // SCH):
                        ps_o = psA.tile([P, SCH], F32, tag="psA", name="pso")
                        for m in range(MB):
                            nc.tensor.matmul(
                                ps_o[:], atn_sb[:, m, t * P:(t + 1) * P],
                                wo_sb[:, m, oc * SCH:(oc + 1) * SCH],
                                start=(m == 0), stop=(m == MB - 1))
                        ot = ostage.tile([P, SCH], F32, tag="ot")
                        nc.vector.tensor_copy(ot[:], ps_o[:])
                        nc.gpsimd.dma_start(
                            outp[t * P:(t + 1) * P, oc * SCH:(oc + 1) * SCH], ot[:])

            def norm_head(h, c, ps_at):
                # AT[0:64] *= broadcast(1/l);  l = ps_at row 64.
                # approx_fast mishandles partition-offset inputs: stage the
                # l row to partition 0 first.
                m, po = h // 2, (h % 2) * HEAD_DIM
                lrow = small.tile([1, SCH], F32, tag="lrow")
                nc.vector.tensor_copy(lrow[:], ps_at[HEAD_DIM:HEAD_DIM + 1, :])
                linv = small.tile([1, SCH], F32, tag="linv")
                nc.vector.reciprocal_approx_fast(out=linv[:], in_=lrow[:])
                lbc = small.tile([HEAD_DIM, SCH], F32, tag="lbc")
                nc.gpsimd.partition_broadcast(lbc[:], linv[:])
                nc.vector.tensor_mul(
                    atn_sb[po:po + HEAD_DIM, m, c * SCH:(c + 1) * SCH],
                    ps_at[0:HEAD_DIM, :], lbc[:])

            qk_proj(0)
            v_proj(0)

            # ---- attention: chunk-major, head pairs, PE filler interleaved ----
            for c in range(NCH):
                nkt = 4 * (c + 1)  # causal: k-tiles 0..nkt-1

                def qoff(kt):
                    # diagonal k-tile j only needs q in [128j, 512)
                    return max(kt - 4 * c, 0) * P

                for hp in range(HPC // 2):
                    atts = []
                    for kt in range(nkt):
                        qo = qoff(kt)
                        ps_s = psS.tile([P, 2, SCH], F32, tag="psS")
                        for hh in range(2):
                            h = 2 * hp + hh
                            m, po = h // 2, (h % 2) * HEAD_DIM
                            nc.tensor.matmul(
                                ps_s[:, hh, qo:],
                                kT_sb[po:po + HEAD_DIM, m, kt * P:(kt + 1) * P],
                                qT_sb[po:po + HEAD_DIM, m,
                                      c * SCH + qo:(c + 1) * SCH],
                                start=True, stop=True)
                        att = attnp.tile([P, 2, SCH], BF16, tag="attn")
                        nc.scalar.activation(att[:, :, qo:],
                                             ps_s[:, :, qo:], Exp, scale=0.125)
                        j = kt - 4 * c
                        if j >= 0:  # diagonal tiles: apply causal mask
                            for hh in range(2):
                                nc.vector.tensor_mul(
                                    att[:, hh, qo:], att[:, hh, qo:],
                                    mk_sb[:, j * SCH + qo:(j + 1) * SCH])
                        atts.append(att)
                    for hh in range(2):
                        h = 2 * hp + hh
                        ps_at = psO.tile([HEAD_DIM + 1, SCH], F32, tag="psO",
                                         name="ps_at")
                        for kt in range(nkt):
                            qo = qoff(kt)
                            nc.tensor.matmul(
                                ps_at[:, qo:], v_sb[:, kt, h, :],
                                atts[kt][:, hh, qo:],
                                start=(kt == 0), stop=(kt == nkt - 1))
                        norm_head(h, c, ps_at)

                    # PE filler between head pairs / chunks: projections for
                    # upcoming chunks + output projection for finished ones.
                    if hp == 0:
                        if c + 1 < NCH:
                            qk_proj(c + 1)
                    else:
                        if c + 1 < NCH:
                            v_proj(c + 1)
                        if c >= 1:
                            wo_proj(c - 1)
            wo_proj(NCH - 1)

    nc.compile()
    return nc


def _get_nc():
    if "nc" not in _CACHE:
        _CACHE["nc"] = _build()
    return _CACHE["nc"]


def _mask_const():
    # mask[j][k, q] = 1.0 iff q >= j*128 + k, for diagonal k-tile offset j
    q = np.arange(SCH)[None, :]
    k = np.arange(P)[:, None]
    blocks = [(q >= j * P + k).astype(ml_dtypes.bfloat16) for j in range(4)]
    return np.concatenate(blocks, axis=1)  # [128, 2048]


def _tile_xt(x_t):
    # [D_MODEL, S] -> [IT, NCH, 128, 512] contiguous tiles
    return np.ascontiguousarray(
        x_t.reshape(IT, P, NCH, SCH).transpose(0, 2, 1, 3))


def _kernel_numpy(query, key, value, mask, Wq, Wk, Wv, Wo, bo):
    # exact f32 fallback for non-causal masks
    q = (query @ Wq.T).reshape(B, S, NUM_HEADS, HEAD_DIM).transpose(0, 2, 1, 3)
    k = (key @ Wk.T).reshape(B, S, NUM_HEADS, HEAD_DIM).transpose(0, 2, 1, 3)
    v = (value @ Wv.T).reshape(B, S, NUM_HEADS, HEAD_DIM).transpose(0, 2, 1, 3)
    s = np.einsum("bhqd,bhkd->bhqk", q, k) / np.sqrt(np.float32(HEAD_DIM))
    s = np.where(np.asarray(mask), s, -np.inf)
    s = s - s.max(axis=-1, keepdims=True)
    e = np.exp(s)
    a = e / e.sum(axis=-1, keepdims=True)
    o = np.einsum("bhqk,bhkd->bhqd", a, v).transpose(0, 2, 1, 3)
    return (o.reshape(B, S, D_MODEL) @ Wo.T + bo).astype(np.float32)


def kernel(query, key, value, mask, Wq, Wk, Wv, Wo, bo):
    from concourse.bass_utils import run_bass_kernel_spmd

    m = np.asarray(mask).astype(bool)
    expect = np.tril(np.ones((S, S), dtype=bool))
    if m.size != S * S or not np.array_equal(m.reshape(S, S), expect):
        args = [np.asarray(a, np.float32) for a in
                (query, key, value)] + [mask] + [
                np.asarray(a, np.float32) for a in (Wq, Wk, Wv, Wo, bo)]
        return _kernel_numpy(*args)

    nc = _get_nc()
    bf = ml_dtypes.bfloat16

    xq_t = [_tile_xt(np.asarray(query)[b].T.astype(bf)) for b in range(B)]
    xk_t = [_tile_xt(np.asarray(key)[b].T.astype(bf)) for b in range(B)]
    xv_t = [_tile_xt(np.asarray(value)[b].T.astype(bf)) for b in range(B)]
    WqT = np.ascontiguousarray(np.asarray(Wq).T).astype(bf)  # [D, D] cols = out dim
    WkT = np.ascontiguousarray(np.asarray(Wk).T).astype(bf)
    WvT = np.ascontiguousarray(np.asarray(Wv).T).astype(bf)
    WoT = np.ascontiguousarray(np.asarray(Wo).T).astype(bf)
    mk = _mask_const()

    in_maps = []
    for core in range(N_CORES):
        b, g = core // GROUPS, core % GROUPS
        hsl = slice(g * DLOC, (g + 1) * DLOC)
        in_maps.append({
            "xq_t": xq_t[b], "xk_t": xk_t[b], "xv_t": xv_t[b],
            "wq_t": np.ascontiguousarray(WqT[:, hsl]),
            "wk_t": np.ascontiguousarray(WkT[:, hsl]),
            "wv_t": np.ascontiguousarray(WvT[:, hsl]),
            "wo_t": np.ascontiguousarray(WoT[hsl, :]),
            "mask": mk,
        })

    res = run_bass_kernel_spmd(nc, in_maps, core_ids=list(range(N_CORES)))
    _CACHE["last_result"] = res

    out = np.zeros((B, S, D_MODEL), np.float32)
    for core in range(N_CORES):
        out[core // GROUPS] += res.results[core]["outp"]
    out += np.asarray(bo, np.float32)[None, None, :]
    return out
